# revision 13
# baseline (speedup 1.0000x reference)
"""Causal single-head attention on 8 Trainium2 NeuronCores.

Reference computation (per batch b of 16):
    q = x @ Wq; k = x @ Wk; v = x @ Wv        # x [2048, 512], W* [512, 64]
    out = softmax_causal(q @ k.T / 8) @ v     # out [2048, 64]

Sharding: data-parallel over batch, 2 batches per core, weights replicated.

Per-core kernel (batch-local b in {0,1}), bf16 matmul inputs with fp32
PSUM accumulation (rel-err ~5e-3, tolerance 2e-2):
  - host supplies xT = x[b].T in bf16; one SBUF tile [128, ND, T] per
    batch so a single strided DMA covers all four 128-deep D-tiles
  - qT/kT: psum[0:64]=qT, psum[64:128]=kT via packed lhsT [Wq|Wk]; for
    token chunk 0 ALSO the swap [Wk|Wq] -> kq (kT low, qT high) so the
    first attention chunk needs no SBUF-shift DMAs at all
  - score matmuls are ROW-TILED: contraction is only HD=64 deep, so two
    k-blocks run CONCURRENTLY in the 128x128 PE array (tile_position
    (0,0)/(64,0) auto-derived from AP base partitions) -> 2x throughput.
    Array rows 0:64 read kT from kq/klo + qT from qk; rows 64:128 read
    kT resident in qk[64:128] + qT duplicated into qd by SBUF DMA
  - scores TRANSPOSED: ST[k, q] -> psum [128, 1024] per block pair
    (first half: blocks 0..njb/2-1; second half: njb/2..njb-1)
  - v projection COL-TILED across the two batches (vT_b0 -> psum rows
    0:64, vT_b1 -> rows 64:128, concurrent), then PE transposes ROW-
    TILED across batches into v natural, packed v1[., j, .] = [v_j | 1]
    (the ones column makes PV emit the softmax denominator for free)
  - exp: ACT from psum -> bf16 ptil; in the batch-1 phase (no projection
    work left) each [128,1024] slab is SPLIT: ACT does [.,0:640], DVE
    does [.,640:1024] via the Schraudolph bf16 trick
    (int16(s*16*log2(e) + 16250.2) bit-pattern ~= bf16 exp(s/8); the
    diffuse softmax averages the ~2% sawtooth error to ~0.1%)
  - causal: k-blocks above the diagonal skipped; diagonal blocks get a
    triangular mask multiply and suffix-sliced matmuls
  - oT[65, 512] accumulates [v|1].T @ p~ over k-blocks in psum; row 64
    is the denominator l; the UNNORMALIZED [65, T] goes out and the
    host does out = o / l (+ final transpose)
  - DMA issue only on the two HWDGE rings (sync + scalar), ordered so
    the first 512 tokens of both batches land first
"""

import sys

sys.path.insert(0, "/opt/trn_rl_repo")

import numpy as np
import ml_dtypes

B, T, D, HD = 16, 2048, 512, 64
NCORES = 8
BPC = B // NCORES          # batches per core
NQ = T // 512              # 512-wide q chunks per batch
NJ = T // 128              # 128-wide k blocks per batch
ND = D // 128              # 128-deep contraction tiles

SCH_A = 16.0 * np.log2(np.e)           # 128 * log2(e) * (1/sqrt(HD))
SCH_B = 16256.0 - 128.0 * 0.045       # 127*2^7 minus Schraudolph centering

_cache = {}


def _build_nc():
    import concourse.bacc as bacc
    import concourse.mybir as mybir
    import concourse.tile as tile

    F32 = mybir.dt.float32
    BF16 = mybir.dt.bfloat16
    I16 = mybir.dt.int16
    AF = mybir.ActivationFunctionType
    ALU = mybir.AluOpType

    nc = bacc.Bacc("TRN2", target_bir_lowering=False, debug=False)

    xt_d = nc.dram_tensor("xt", [BPC, D, T], BF16, kind="ExternalInput")
    wqk_d = nc.dram_tensor("wqk", [128, ND * 128], BF16, kind="ExternalInput")
    wkq_d = nc.dram_tensor("wkq", [128, ND * 128], BF16, kind="ExternalInput")
    wv_d = nc.dram_tensor("wv", [128, ND * HD], BF16, kind="ExternalInput")
    ident_d = nc.dram_tensor("ident", [128, 64], BF16, kind="ExternalInput")
    mask_d = nc.dram_tensor("mask", [128, 128], BF16, kind="ExternalInput")
    # Schraudolph bias plane: C[k, u] = SCH_B - 10000*(k > u); columns
    # 512:1024 are all-SCH_B (the non-masked case)
    schc_d = nc.dram_tensor("schc", [128, 1024], F32, kind="ExternalInput")
    ot_d = nc.dram_tensor("ot", [BPC, HD + 1, T], F32, kind="ExternalOutput")

    with tile.TileContext(nc) as tc:
        with (
            tc.tile_pool(name="const", bufs=1) as cpool,
            tc.tile_pool(name="xt", bufs=1) as xtpool,
            tc.tile_pool(name="qk", bufs=2) as qkpool,
            tc.tile_pool(name="kq", bufs=2) as kqpool,
            tc.tile_pool(name="qd", bufs=2) as qdpool,
            tc.tile_pool(name="klo", bufs=2) as klopool,
            tc.tile_pool(name="vt", bufs=2) as vtpool,
            tc.tile_pool(name="v1", bufs=2) as v1pool,
            tc.tile_pool(name="pt", bufs=3) as ptpool,
            tc.tile_pool(name="ob", bufs=2) as obpool,
            tc.tile_pool(name="st", bufs=2, space="PSUM") as stpool,
            tc.tile_pool(name="otp", bufs=2, space="PSUM") as otpool,
            tc.tile_pool(name="aux", bufs=2, space="PSUM") as auxpool,
        ):
            # warm the exp table set on ACT immediately (no DMA dependency)
            scratch = cpool.tile([1, 8], F32, tag="scratch")
            nc.vector.memset(scratch[:], 0.0)
            scratch2 = cpool.tile([1, 8], F32, tag="scratch2")
            nc.scalar.activation(scratch2[:], scratch[:], AF.Exp)

            # ---- input DMAs, ordered by need; sync + scalar HWDGE rings ----
            xtc = {}
            for b in range(BPC):
                xtc[b] = xtpool.tile([128, ND, T], BF16, tag=f"xt{b}", name=f"xt{b}")
            xsrc = {
                b: xt_d[b].rearrange("(d p) t -> p d t", p=128) for b in range(BPC)
            }
            nc.sync.dma_start(xtc[0][:, :, 0:512], xsrc[0][:, :, 0:512])
            wqk = cpool.tile([128, ND, 128], BF16, tag="wqk")
            nc.scalar.dma_start(wqk[:], wqk_d[:].rearrange("p (d c) -> p d c", d=ND))
            wkq = cpool.tile([128, ND, 128], BF16, tag="wkq")
            nc.scalar.dma_start(wkq[:], wkq_d[:].rearrange("p (d c) -> p d c", d=ND))
            nc.scalar.dma_start(xtc[1][:, :, 0:512], xsrc[1][:, :, 0:512])
            ident = cpool.tile([128, 64], BF16, tag="ident")
            nc.sync.dma_start(ident[:], ident_d[:])
            mask = cpool.tile([128, 128], BF16, tag="mask")
            nc.sync.dma_start(mask[:], mask_d[:])
            wv = cpool.tile([128, ND, HD], BF16, tag="wv")
            nc.scalar.dma_start(wv[:], wv_d[:].rearrange("p (d c) -> p d c", d=ND))
            schc = cpool.tile([128, 1024], F32, tag="schc")
            nc.scalar.dma_start(schc[:], schc_d[:])
            for lo, hi in ((512, 1024), (1024, T)):
                for b in range(BPC):
                    nc.scalar.dma_start(
                        xtc[b][:, :, lo:hi], xsrc[b][:, :, lo:hi]
                    )

            qks, kqs, qds, klos, v1s = {}, {}, {}, {}, {}
            for b in range(BPC):
                qks[b] = qkpool.tile([128, T], BF16, tag="qk", name=f"qk{b}")
                kqs[b] = kqpool.tile([128, 512], BF16, tag="kq", name=f"kq{b}")
                qds[b] = qdpool.tile([128, T], BF16, tag="qd", name=f"qd{b}")
                klos[b] = klopool.tile([64, 512], BF16, tag="klo", name=f"klo{b}")
                v1s[b] = v1pool.tile(
                    [128, NJ, HD + 1], BF16, tag="v1", name=f"v1{b}"
                )
                nc.vector.memset(v1s[b][:, :, HD:HD + 1], 1.0)
            vt2 = vtpool.tile([128, T], BF16, tag="vt", name="vt2")

            def emit_qkproj(b, Q):
                """qT/kT for tokens [512Q, 512Q+512) of batch b. Chunk 0
                additionally projects the swapped pack [Wk|Wq] -> kq so the
                first attention chunk needs no partition-shift DMAs; later
                chunks shift qT to partitions 64:128 (qd) by SBUF DMA, and
                chunk 1 also drops kT blocks 4-7 to partitions 0:64 (klo)."""
                s = slice(512 * Q, 512 * (Q + 1))
                qk = qks[b]
                p = auxpool.tile([128, 512], F32, tag="aux", name="pqk")
                for d in range(ND):
                    nc.tensor.matmul(
                        p[:], wqk[:, d, :], xtc[b][:, d, s],
                        start=(d == 0), stop=(d == ND - 1),
                    )
                nc.scalar.copy(qk[:, s], p[:])
                if Q == 0:
                    pk = auxpool.tile([128, 512], F32, tag="aux", name="pkq")
                    for d in range(ND):
                        nc.tensor.matmul(
                            pk[:], wkq[:, d, :], xtc[b][:, d, s],
                            start=(d == 0), stop=(d == ND - 1),
                        )
                    nc.scalar.copy(kqs[b][:], pk[:])
                else:
                    nc.sync.dma_start(qds[b][64:128, s], qk[0:64, s])
                    if Q == 1:
                        nc.sync.dma_start(klos[b][0:64, :], qk[64:128, s])

            def emit_vpair(Q):
                """v for tokens [512Q, 512Q+512) of BOTH batches: projection
                col-tiled (b0 -> psum rows 0:64, b1 -> rows 64:128), PE
                transposes row-tiled, both pairs running concurrently."""
                s = slice(512 * Q, 512 * (Q + 1))
                pvv = auxpool.tile([128, 512], F32, tag="aux", name="pvv")
                for d in range(ND):
                    nc.tensor.matmul(
                        pvv[0:64, :], wv[:, d, :], xtc[0][:, d, s],
                        start=(d == 0), stop=(d == ND - 1),
                    )
                    nc.tensor.matmul(
                        pvv[64:128, :], wv[:, d, :], xtc[1][:, d, s],
                        start=(d == 0), stop=(d == ND - 1),
                    )
                nc.vector.tensor_copy(vt2[:, s], pvv[:])
                for t2 in range(2 * Q, 2 * Q + 2):
                    p2a = auxpool.tile([128, 128], BF16, tag="aux", name="p2a")
                    p2b = auxpool.tile([128, 128], BF16, tag="aux", name="p2b")
                    for tt in range(2):
                        ts_ = slice(128 * (2 * t2 + tt), 128 * (2 * t2 + tt + 1))
                        nc.tensor.transpose(
                            p2a[:, 64 * tt:64 * (tt + 1)],
                            vt2[0:64, ts_], ident[0:64, :],
                        )
                        nc.tensor.transpose(
                            p2b[:, 64 * tt:64 * (tt + 1)],
                            vt2[64:128, ts_], ident[64:128, :],
                        )
                    nc.vector.tensor_copy(
                        v1s[0][:, 2 * t2:2 * t2 + 2, 0:HD],
                        p2a[:].rearrange("p (a c) -> p a c", a=2),
                    )
                    nc.vector.tensor_copy(
                        v1s[1][:, 2 * t2:2 * t2 + 2, 0:HD],
                        p2b[:].rearrange("p (a c) -> p a c", a=2),
                    )

            def emit_attn_q(b, Q):
                """One query chunk. Score matmuls are row-tiled pairs: block
                g (rows 0:64, kT from kq/klo) runs concurrently with block
                njb/2+g (rows 64:128, kT in qk[64:128] vs qd/kq). Each pair's
                softmax runs on BOTH elementwise engines in parallel: ACT
                exps block g into ptil_a, DVE runs a fused Schraudolph-exp +
                causal-mask scalar_tensor_tensor for block njb/2+g into
                ptil_b. PV skewed one pair behind."""
                qk, v1 = qks[b], v1s[b]
                pot = otpool.tile([HD + 1, 512], F32, tag="ot", name="pot")
                njb = 4 * (Q + 1)          # causal k-blocks for this chunk
                half = njb // 2
                jlast = njb - 1
                pending = None

                def w0_of(j):
                    return 128 * (j - 4 * Q) if j >= 4 * Q else 0

                def emit_pv(pa, pb, j1, j2):
                    for p_tile, j in ((pa, j1), (pb, j2)):
                        w0 = w0_of(j)
                        nc.tensor.matmul(
                            pot[:, w0:512],
                            v1[:, j, :],
                            p_tile[:, w0:512],
                            start=(j == 0),
                            stop=(j == jlast),
                        )

                for g in range(half):
                    j1, j2 = g, half + g
                    pst = stpool.tile([128, 1024], F32, tag="st", name="pst")
                    w1, w2 = w0_of(j1), w0_of(j2)
                    klo_src = (
                        kqs[b][0:64, 128 * j1:128 * (j1 + 1)]
                        if j1 < 4 else
                        klos[b][0:64, 128 * (j1 - 4):128 * (j1 - 3)]
                    )
                    qhi_src = (
                        kqs[b][64:128, w2:512] if Q == 0 else
                        qds[b][64:128, 512 * Q + w2:512 * (Q + 1)]
                    )
                    nc.tensor.matmul(
                        pst[:, w1:512],
                        klo_src,
                        qk[0:64, 512 * Q + w1:512 * (Q + 1)],
                        start=True, stop=True,
                    )
                    nc.tensor.matmul(
                        pst[:, 512 + w2:1024],
                        qk[64:128, 128 * j2:128 * (j2 + 1)],
                        qhi_src,
                        start=True, stop=True,
                    )
                    pta = ptpool.tile([128, 512], BF16, tag="pta", name="pta")
                    ptb = ptpool.tile([128, 512], BF16, tag="ptb", name="ptb")
                    nc.scalar.activation(
                        pta[:, w1:512], pst[:, w1:512], AF.Exp,
                        scale=1.0 / np.sqrt(HD),
                    )
                    if j1 >= 4 * Q:  # only chunk 0: diagonal block in idx0
                        nc.vector.tensor_mul(
                            pta[:, w1:w1 + 128], pta[:, w1:w1 + 128], mask[:]
                        )
                    # fused Schraudolph exp + causal mask: the bias plane is
                    # SCH_B - 10000 in masked spots -> bf16 bit pattern ~ 0
                    u0 = 0 if j2 >= 4 * Q else 512
                    nc.vector.scalar_tensor_tensor(
                        ptb[:, w2:512].bitcast(I16),
                        pst[:, 512 + w2:1024],
                        SCH_A,
                        schc[:, u0:u0 + 512 - w2],
                        ALU.mult, ALU.add,
                    )
                    if pending is not None:
                        emit_pv(*pending)
                    pending = (pta, ptb, j1, j2)
                emit_pv(*pending)

                # unnormalized o (rows 0:64) + denominator l (row 64) out;
                # the host divides
                osb = obpool.tile([HD + 1, 512], F32, tag="ob", name="osb")
                nc.scalar.copy(osb[:], pot[:])
                nc.sync.dma_start(ot_d[b, :, 512 * Q:512 * (Q + 1)], osb[:])

            # ---- emission schedule: projections one chunk ahead, the two
            # batches' attention chunks interleaved so every chunk's shift
            # DMAs were issued a full chunk earlier ----
            for Q in range(NQ):
                emit_qkproj(0, Q)
                emit_qkproj(1, Q)
                emit_vpair(Q)
                emit_attn_q(0, Q)
                emit_attn_q(1, Q)

    nc.compile()
    return nc


def _get_nc():
    if "nc" not in _cache:
        _cache["nc"] = _build_nc()
    return _cache["nc"]


def _pack_w(w):
    # [512, C] -> partition-major [128, ND*C]: out[p, d*C+c] = w[128d+p, c]
    C = w.shape[1]
    return np.ascontiguousarray(
        w.reshape(ND, 128, C).transpose(1, 0, 2).reshape(128, ND * C)
    ).astype(ml_dtypes.bfloat16)


def kernel(x, Wq, Wk, Wv, _trace=False, _trace_kwargs=None):
    from concourse.bass_utils import run_bass_kernel_spmd

    x = np.asarray(x, dtype=np.float32)
    Wq = np.asarray(Wq, dtype=np.float32)
    Wk = np.asarray(Wk, dtype=np.float32)
    Wv = np.asarray(Wv, dtype=np.float32)

    nc = _get_nc()

    bf16 = ml_dtypes.bfloat16
    wqk = _pack_w(np.concatenate([Wq, Wk], axis=1))
    wkq = _pack_w(np.concatenate([Wk, Wq], axis=1))
    wv = _pack_w(Wv)
    eye = np.eye(64, dtype=np.float32)
    ident = np.concatenate([eye, eye], axis=0).astype(bf16)
    mask = np.triu(np.ones((128, 128), dtype=np.float32)).astype(bf16)
    kk, uu = np.meshgrid(np.arange(128), np.arange(1024), indexing="ij")
    schc = (SCH_B - 10000.0 * (kk > uu)).astype(np.float32)

    in_maps = []
    for c in range(NCORES):
        xt = np.ascontiguousarray(
            x[BPC * c:BPC * (c + 1)].transpose(0, 2, 1).astype(bf16)
        )
        in_maps.append(
            {
                "xt": xt, "wqk": wqk, "wkq": wkq, "wv": wv,
                "ident": ident, "mask": mask, "schc": schc,
            }
        )

    kwargs = dict(_trace_kwargs or {})
    res = run_bass_kernel_spmd(
        nc, in_maps, list(range(NCORES)), trace=_trace, **kwargs
    )

    out = np.empty((B, T, HD), dtype=np.float32)
    for c in range(NCORES):
        ot = res.results[c]["ot"]  # [BPC, HD+1, T] unnormalized + denominator
        o = ot[:, 0:HD, :] / ot[:, HD:HD + 1, :]
        out[BPC * c:BPC * (c + 1)] = o.transpose(0, 2, 1)
    if _trace:
        _cache["last_results"] = res
    return out
